# revision 15
# baseline (speedup 1.0000x reference)
"""Trainium2 Bass kernel for nn_Attention_42279658062045 (gnn_message_passing).

Computes, for each of B=200000 nodes:
    simi   = exp(-source_distance^2 / 2)                  [B, K]
    weight = softmax(simi @ kernel + bias, axis=-1)       [B, K]
    mean   = einsum('bk,bkd->bd', weight, context)        [B, D]

Sharding: pure data parallel over the node axis B across 8 NeuronCores;
kernel/bias replicated; no cross-device communication.

Per-core dataflow (B_LOCAL = 25000 rows, tiles of 128 rows, f32 throughout):
  - HWDGE streams context in 4-tile (~4 MB) chunks (dominant HBM traffic).
  - PE: transpose of simi tiles; logits = simi @ kernel + bias via two
    accumulating matmuls (ones-row stationary adds the bias).
  - ACT: batched square+exp of all distances, PSUM->SBUF copy of simi^T,
    exp(logits) with accum_out giving the softmax denominator, the
    weighted product for ACT_SLABS of the 30 k-slabs (per-partition
    scale), and the final 1/Z normalization of each output tile.
  - DVE: reciprocal, weighted product for the other k-slabs (plain
    tensor_tensor against a broadcast of exp(logits)), and the full-30
    k-reduction over the interleaved product.

The product tensor uses an interleaved layout [d_hi(32), k(30), d_lo(2)]
(flat addr = d_hi*60 + k*2 + d_lo) so the k-reduction reads at 8-byte
stride, which the DVE streams at near-full rate (256-byte strides cost
~1.6x). GPSIMD is deliberately unused: measured on hardware, any
substantial DVE op that overlaps a running GPSIMD op stalls until the
GPSIMD op completes (shared SBUF port), so offloading fold work to
GPSIMD subtracts directly from DVE throughput instead of adding
parallelism.
"""

import numpy as np

N_CORES = 8
B, K, D = 200000, 30, 64
B_LOCAL = B // N_CORES  # 25000
P = 128
CT = 4          # tiles per context DMA chunk
IL = 2          # product interleave: [d_hi(32), k(30), d_lo(IL)]
DH = D // IL    # 32
ACT_SLABS = 7   # k-slabs whose product is computed on the scalar engine
DVE_SLABS = K - ACT_SLABS

_CACHE = {}


def _build():
    import concourse.bacc as bacc
    import concourse.tile as tile
    from concourse import mybir
    from concourse.masks import make_identity

    fp32 = mybir.dt.float32
    AF = mybir.ActivationFunctionType

    nc = bacc.Bacc("TRN2", target_bir_lowering=False, debug=False,
                   num_devices=N_CORES)

    dist = nc.dram_tensor("source_distance", [B_LOCAL, K], fp32,
                          kind="ExternalInput").ap()
    ctx_d = nc.dram_tensor("context", [B_LOCAL, K, D], fp32,
                           kind="ExternalInput").ap()
    kern = nc.dram_tensor("kernel", [K, K], fp32, kind="ExternalInput").ap()
    bias = nc.dram_tensor("bias", [K], fp32, kind="ExternalInput").ap()
    out = nc.dram_tensor("out", [B_LOCAL, D], fp32, kind="ExternalOutput").ap()

    n_full = B_LOCAL // P          # 195 full tiles
    rem = B_LOCAL - n_full * P     # 40 leftover rows

    dist_v = dist[:n_full * P, :].rearrange("(n p) k -> p n k", p=P)
    ctx_v = ctx_d[:n_full * P].rearrange("(n p) k d -> p n (k d)", p=P)
    out_v = out[:n_full * P, :].rearrange("(n p) d -> p n d", p=P)

    with tile.TileContext(nc) as tc:
        from contextlib import ExitStack
        with ExitStack() as st:
            consts = st.enter_context(tc.tile_pool(name="consts", bufs=1))
            big = st.enter_context(tc.tile_pool(name="big", bufs=1))
            ctxp = st.enter_context(tc.tile_pool(name="ctx", bufs=2))
            prodp = st.enter_context(tc.tile_pool(name="prod", bufs=3))
            small = st.enter_context(tc.tile_pool(name="small", bufs=3))
            rzp = st.enter_context(tc.tile_pool(name="rz", bufs=10))
            psum_t = st.enter_context(
                tc.tile_pool(name="psumT", bufs=2, space="PSUM"))
            psum_l = st.enter_context(
                tc.tile_pool(name="psumL", bufs=2, space="PSUM"))

            ident = consts.tile([P, P], fp32)
            make_identity(nc, ident)
            kern_s = consts.tile([K, K], fp32)
            nc.sync.dma_start(out=kern_s, in_=kern)
            bias_s = consts.tile([1, K], fp32)
            nc.sync.dma_start(out=bias_s, in_=bias.unsqueeze(0))
            ones_s = consts.tile([1, P], fp32)
            nc.vector.memset(ones_s, 1.0)

            # All distances for the full tiles; squared+exp'd in place.
            simi_all = big.tile([P, n_full, K], fp32)
            nc.sync.dma_start(out=simi_all, in_=dist_v)
            nc.scalar.activation(out=simi_all, in_=simi_all, func=AF.Square)
            nc.scalar.activation(out=simi_all, in_=simi_all, func=AF.Exp,
                                 scale=-0.5)

            # Staged output for the full tiles (one big DMA at the end).
            mean_all = big.tile([P, n_full, D], fp32)

            def softmax_exp(simi_ap, expw_ap, rows):
                """simi [rows, K] -> expw (unnormalized softmax numerator)
                written to expw_ap, returns rz = 1/sum(expw) [rows, 1].

                No DVE op here reads a per-partition scalar operand
                (TensorScalarPtr): those hard-block for the whole duration
                of any concurrent GPSIMD op (shared SBUF port, measured
                6.8us stalls), so normalization is applied on the scalar
                engine at the very end instead.
                """
                simiT_p = psum_t.tile([K, P], fp32, tag="simiT_p")
                nc.tensor.transpose(out=simiT_p[:, :rows], in_=simi_ap,
                                    identity=ident[:rows, :rows])
                simiT_s = small.tile([K, P], fp32, tag="simiT_s")
                nc.scalar.copy(out=simiT_s[:, :rows], in_=simiT_p[:, :rows])

                logits_p = psum_l.tile([P, K], fp32, tag="logits_p")
                nc.tensor.matmul(out=logits_p[:rows, :],
                                 lhsT=simiT_s[:, :rows], rhs=kern_s,
                                 start=True, stop=False)
                nc.tensor.matmul(out=logits_p[:rows, :],
                                 lhsT=ones_s[:, :rows], rhs=bias_s,
                                 start=False, stop=True)

                zsum = small.tile([P, 1], fp32, tag="zsum")
                nc.scalar.activation(out=expw_ap, in_=logits_p[:rows, :],
                                     func=AF.Exp, accum_out=zsum[:rows, :])
                rz = rzp.tile([P, 1], fp32, tag="rz")
                nc.vector.reciprocal(out=rz[:rows, :], in_=zsum[:rows, :])
                return rz

            # Remainder rows (partial tile): simple all-DVE path, emitted
            # first so its serial chain overlaps the main pipeline.
            if rem:
                simi_r = small.tile([P, K], fp32, tag="simi_r")
                nc.sync.dma_start(out=simi_r[:rem, :], in_=dist[n_full * P:, :])
                nc.scalar.activation(out=simi_r[:rem, :], in_=simi_r[:rem, :],
                                     func=AF.Square)
                nc.scalar.activation(out=simi_r[:rem, :], in_=simi_r[:rem, :],
                                     func=AF.Exp, scale=-0.5)
                ctx_r = ctxp.tile([P, K * D], fp32, tag="ctx")
                nc.sync.dma_start(
                    out=ctx_r[:rem, :],
                    in_=ctx_d[n_full * P:].rearrange("b k d -> b (k d)"))
                expw_r = small.tile([P, K], fp32, tag="expw_r")
                rz_r = softmax_exp(simi_r[:rem, :], expw_r[:rem, :], rem)
                prod_r = prodp.tile([P, K, D], fp32, tag="prod_r")
                ctx3r = ctx_r[:rem, :].rearrange("p (k d) -> p k d", k=K)
                nc.vector.tensor_mul(
                    out=prod_r[:rem], in0=ctx3r,
                    in1=expw_r[:rem, :].unsqueeze(2).broadcast_to([rem, K, D]))
                mean_r = small.tile([P, D], fp32, tag="mean_r")
                nc.vector.reduce_sum(
                    out=mean_r[:rem, :],
                    in_=prod_r[:rem].rearrange("p k d -> p d k"),
                    axis=mybir.AxisListType.X)
                nc.scalar.mul(out=mean_r[:rem, :], in_=mean_r[:rem, :],
                              mul=rz_r[:rem, :])
                nc.sync.dma_start(out=out[n_full * P:, :], in_=mean_r[:rem, :])

            # Full tiles: context DMA in CT-tile chunks; per-tile products
            # (interleaved), full-30 DVE reduce, ACT normalize.
            ctx_tile = None
            for t in range(n_full):
                cc, lane = divmod(t, CT)
                if lane == 0:
                    ctx_tile = ctxp.tile([P, CT, K * D], fp32, tag="ctx")
                    lo = cc * CT
                    cn = min(CT, n_full - lo)
                    nc.sync.dma_start(out=ctx_tile[:, :cn, :],
                                      in_=ctx_v[:, lo:lo + cn, :])
                ctx3 = ctx_tile[:, lane, :].rearrange("p (k d) -> p k d", k=K)

                ew = small.tile([P, K], fp32, tag="ew")
                rz = softmax_exp(simi_all[:, t, :], ew, P)

                prod = prodp.tile([P, DH, K, IL], fp32, tag="prod")
                # DVE: product for slabs [0, DVE_SLABS), interleaved out
                nc.vector.tensor_mul(
                    out=prod.rearrange("p h k l -> p k h l")[:, :DVE_SLABS],
                    in0=ctx3[:, :DVE_SLABS, :],
                    in1=ew[:, :DVE_SLABS].unsqueeze(2).broadcast_to(
                        [P, DVE_SLABS, D]))
                # ACT: product for the last ACT_SLABS slabs
                for k in range(DVE_SLABS, K):
                    nc.scalar.mul(out=prod[:, :, k, :],
                                  in_=ctx3[:, k, :], mul=ew[:, k:k + 1])

                # DVE: full-30 reduce at 8-byte stride; ACT: normalize
                nc.vector.reduce_sum(
                    out=mean_all[:, t, :].rearrange("p (h l) -> p h l", l=IL),
                    in_=prod.rearrange("p h k l -> p h l k"),
                    axis=mybir.AxisListType.X)
                nc.scalar.mul(out=mean_all[:, t, :], in_=mean_all[:, t, :],
                              mul=rz)
                # stream the staged output out in 8-tile slices
                if (t + 1) % 8 == 0 or t == n_full - 1:
                    g0 = (t // 8) * 8
                    nc.sync.dma_start(out=out_v[:, g0:t + 1, :],
                                      in_=mean_all[:, g0:t + 1, :])

    nc.compile()
    return nc


def _get_nc():
    if "nc" not in _CACHE:
        _CACHE["nc"] = _build()
    return _CACHE["nc"]


def kernel(source_distance, context, kernel, bias, _trace=False, _tmpdir=None):
    from concourse.bass_utils import run_bass_kernel_spmd

    nc = _get_nc()

    source_distance = np.ascontiguousarray(source_distance, dtype=np.float32)
    context = np.ascontiguousarray(context, dtype=np.float32)
    kernel = np.ascontiguousarray(kernel, dtype=np.float32)
    bias = np.ascontiguousarray(bias, dtype=np.float32)

    in_maps = []
    for i in range(N_CORES):
        lo, hi = i * B_LOCAL, (i + 1) * B_LOCAL
        in_maps.append({
            "source_distance": source_distance[lo:hi],
            "context": context[lo:hi],
            "kernel": kernel,
            "bias": bias,
        })

    res = run_bass_kernel_spmd(nc, in_maps, list(range(N_CORES)),
                               trace=_trace, tmpdir=_tmpdir)
    out = np.concatenate([res.results[i]["out"] for i in range(N_CORES)], axis=0)
    if _trace:
        _CACHE["last_results"] = res
    return out
